# revision 1
# baseline (speedup 1.0000x reference)
"""FP8 GEMM kernel (MixLinear) for 8 trn2 NeuronCores.

Reference computation:
    s      = max(|x|) / 448                        (global fp32 scalar)
    q_x    = e4m3fn(clip(x / s, +-448))            (OCP e4m3fn)
    q_w    = e4m3fn(clip(w, +-448))                (scale_weight = 1)
    y      = (q_x @ q_w.T) * s + bias              (fp32 accum -> fp16)

Strategy: data-parallel over the 16384 token rows (2048 rows per core).
Host does layout only (transpose so the contraction dim d_in lands on
SBUF partitions, then slice); device does amax, a cross-core AllGather
of the per-core maxima, quantization, DoubleRow fp8 matmul and
scale+bias eviction.

TRN e4m3 tops out at 240 (vs OCP 448), so x is quantized at half scale:
    q_half = trn_e4m3(x * (224/gmax))  ==  ocp_e4m3(x / s) / 2
exactly for all magnitudes >= 2^-6 * s (below that the two grids differ
by one subnormal bit -- negligible).  Weights (|w| <= 1/sqrt(2048)) are
in the range where the TRN and OCP grids agree exactly, so they are
quantized at scale 1.  The output scale is then 2*s = gmax/224.

DoubleRow pairing: adjacent d_in rows (2p, 2p+1) share a PE cell, so
each SBUF partition p loads one contiguous 8KB block of the transposed
operand -- max-rate DMA.
"""

import numpy as np

B, S, D_IN, D_OUT = 2, 8192, 2048, 2048
N_CORES = 8
TOK = B * S                  # 16384
TOK_PC = TOK // N_CORES      # 2048 token rows per core
P = 128
KP = D_IN // (2 * P)         # 8 k-pairs of 256 (DoubleRow granularity)
MT = TOK_PC // P             # 16 token tiles per core
N_TILE = 512
NT = D_OUT // N_TILE         # 4 output column tiles

_compiled = None


def _build():
    import concourse.bacc as bacc
    import concourse.tile as tile
    from concourse import mybir
    from concourse.masks import make_identity

    f16 = mybir.dt.float16
    f32 = mybir.dt.float32
    f8 = mybir.dt.float8e4
    Alu = mybir.AluOpType
    Axis = mybir.AxisListType
    Act = mybir.ActivationFunctionType

    nc = bacc.Bacc("TRN2", target_bir_lowering=False, debug=False,
                   num_devices=N_CORES)

    # xt: x^T shard [d_in, tok_pc]; wt: w^T [d_in, d_out] (replicated)
    xt = nc.dram_tensor("xt", [D_IN, TOK_PC], f16, kind="ExternalInput")
    wt = nc.dram_tensor("wt", [D_IN, D_OUT], f16, kind="ExternalInput")
    bias = nc.dram_tensor("bias", [D_OUT], f16, kind="ExternalInput")
    y = nc.dram_tensor("y", [TOK_PC, D_OUT], f16, kind="ExternalOutput")

    # DRAM bounce buffers for the max AllGather (16 f32 = 64B aligned)
    cc_in = nc.dram_tensor("cc_in", [16], f32)
    cc_out = nc.dram_tensor("cc_out", [16 * N_CORES], f32, addr_space="Shared")

    groups = [list(range(N_CORES))]

    with tile.TileContext(nc) as tc:
        with (
            tc.tile_pool(name="xpool", bufs=KP) as xpool,
            tc.tile_pool(name="qxpool", bufs=KP) as qxpool,
            tc.tile_pool(name="qwpool", bufs=KP) as qwpool,
            tc.tile_pool(name="wstage", bufs=3) as wstage,
            tc.tile_pool(name="small", bufs=1) as small,
            tc.tile_pool(name="ypool", bufs=3) as ypool,
            tc.tile_pool(name="psum", bufs=8, space="PSUM") as psum,
        ):
            # identity for the PE-transpose partition fold (off critical path)
            ident = small.tile([P, P], f32)
            make_identity(nc, ident[:])

            # ---- Phase A: load x^T shard, abs-max as tiles arrive ----
            x_sb = []
            pmax = small.tile([P, KP], f32)
            for j in range(KP):
                t = xpool.tile([P, 2, TOK_PC], f16, tag="xsb")
                src = xt[2 * j * P:(2 * j + 2) * P, :]
                nc.sync.dma_start(t[:], src.rearrange("(p t) m -> p t m", t=2))
                nc.vector.tensor_reduce(
                    out=pmax[:, j:j + 1], in_=t[:], axis=Axis.XY,
                    op=Alu.max, apply_absolute_value=True)
                x_sb.append(t)

            lmax = small.tile([P, 1], f32)
            nc.vector.tensor_reduce(out=lmax[:], in_=pmax[:], axis=Axis.X,
                                    op=Alu.max)
            # fold 128 partitions -> [1, 128] via PE transpose, then reduce
            lmax_t = psum.tile([1, P], f32, tag="ps", name="lmaxt")
            nc.tensor.transpose(lmax_t[:], lmax[:], ident[:])
            lmax16 = small.tile([1, 16], f32)
            nc.vector.memset(lmax16[:], 0.0)
            nc.vector.tensor_reduce(out=lmax16[:, 0:1], in_=lmax_t[:],
                                    axis=Axis.X, op=Alu.max)

            # ---- Phase B: gather per-core maxima, reduce locally ----
            nc.sync.dma_start(cc_in[:], lmax16[:])
            nc.gpsimd.collective_compute(
                "AllGather", Alu.bypass, replica_groups=groups,
                ins=[cc_in.ap().opt()], outs=[cc_out.ap().opt()])
            # every rank block is [lmax, 0 x 15]; max over all 128 = gmax
            gall = small.tile([1, 16 * N_CORES], f32)
            nc.sync.dma_start(gall[:], cc_out[None, :])
            gmax0 = small.tile([1, 1], f32)
            nc.vector.tensor_reduce(out=gmax0[:], in_=gall[:], axis=Axis.X,
                                    op=Alu.max)
            # scale math on partition 0: col0 = inv_half, col1 = out_scale
            sc = small.tile([1, 2], f32)
            nc.vector.reciprocal(sc[:, 0:1], gmax0[:])
            nc.vector.tensor_scalar_mul(sc[:, 0:1], sc[:, 0:1], 224.0)
            nc.vector.tensor_scalar_mul(sc[:, 1:2], gmax0[:], 1.0 / 224.0)
            scales = small.tile([P, 2], f32)
            nc.gpsimd.partition_broadcast(scales[:], sc[:], P)
            inv_half = scales[:, 0:1]
            out_scale = scales[:, 1:2]

            # ---- weights: load w^T, cast to fp8 on ACT (|w| << 240) ----
            qw = []
            for j in range(KP):
                stage = wstage.tile([P, 2, D_OUT], f16, tag="wst")
                src = wt[2 * j * P:(2 * j + 2) * P, :]
                nc.scalar.dma_start(stage[:], src.rearrange("(p t) n -> p t n", t=2))
                qt = qwpool.tile([P, 2, D_OUT], f8, tag="qw")
                nc.scalar.activation(qt[:], stage[:], Act.Copy)
                qw.append(qt)

            # bias broadcast to all partitions
            bias_row = small.tile([1, D_OUT], f16)
            nc.sync.dma_start(bias_row[:], bias[None, :])
            bias_bc = small.tile([P, D_OUT], f16)
            nc.gpsimd.partition_broadcast(bias_bc[:], bias_row[:], P)

            # ---- Phase C: quantize x at half scale ----
            # first 2 token tiles of every k-pair go first (on DVE) so the
            # matmul phase can start while the rest quantizes (DVE/ACT split)
            C0 = 2 * P
            qx = []
            for j in range(KP):
                qt = qxpool.tile([P, 2, TOK_PC], f8, tag="qx")
                nc.vector.tensor_scalar(out=qt[:, :, :C0],
                                        in0=x_sb[j][:, :, :C0],
                                        scalar1=inv_half[:, 0:1],
                                        scalar2=None, op0=Alu.mult)
                qx.append(qt)
            for j in range(KP):
                if j % 2 == 0:
                    nc.vector.tensor_scalar(out=qx[j][:, :, C0:],
                                            in0=x_sb[j][:, :, C0:],
                                            scalar1=inv_half[:, 0:1],
                                            scalar2=None, op0=Alu.mult)
                else:
                    nc.scalar.activation(qx[j][:, :, C0:], x_sb[j][:, :, C0:],
                                         Act.Copy, scale=inv_half[:, 0:1])

            # ---- Phase D: DoubleRow fp8 matmul + fused scale/bias ----
            for mt in range(MT):
                ps = [psum.tile([P, N_TILE], f32, tag="ps", name=f"ps{nt}")
                      for nt in range(NT)]
                for j in range(KP):
                    lhsT = qx[j][:, :, mt * P:(mt + 1) * P]
                    for nt in range(NT):
                        nc.tensor.matmul(
                            ps[nt][:],
                            lhsT,
                            qw[j][:, :, nt * N_TILE:(nt + 1) * N_TILE],
                            start=(j == 0), stop=(j == KP - 1),
                            perf_mode=mybir.MatmulPerfMode.DoubleRow)
                ysb = ypool.tile([P, D_OUT], f16, tag="ysb")
                for nt in range(NT):
                    nc.vector.scalar_tensor_tensor(
                        out=ysb[:, nt * N_TILE:(nt + 1) * N_TILE],
                        in0=ps[nt][:], scalar=out_scale[:, 0:1],
                        in1=bias_bc[:, nt * N_TILE:(nt + 1) * N_TILE],
                        op0=Alu.mult, op1=Alu.add)
                    nc.sync.dma_start(
                        y[mt * P:(mt + 1) * P, nt * N_TILE:(nt + 1) * N_TILE],
                        ysb[:, nt * N_TILE:(nt + 1) * N_TILE])

    nc.compile()
    return nc


def _get_compiled():
    global _compiled
    if _compiled is None:
        _compiled = _build()
    return _compiled


def run(x, weight, bias, **kw):
    """Shard + run on 8 cores; returns (full_output, BassKernelResults)."""
    from concourse.bass_utils import run_bass_kernel_spmd

    nc = _get_compiled()

    x = np.asarray(x, dtype=np.float16)
    weight = np.asarray(weight, dtype=np.float16)
    bias = np.asarray(bias, dtype=np.float16)
    xt = np.ascontiguousarray(x.reshape(TOK, D_IN).T)          # [d_in, tok]
    wt = np.ascontiguousarray(weight.T)                        # [d_in, d_out]
    in_maps = []
    for i in range(N_CORES):
        in_maps.append({
            "xt": np.ascontiguousarray(xt[:, i * TOK_PC:(i + 1) * TOK_PC]),
            "wt": wt,
            "bias": bias,
        })
    res = run_bass_kernel_spmd(nc, in_maps, core_ids=list(range(N_CORES)), **kw)
    out = np.concatenate([res.results[i]["y"] for i in range(N_CORES)], axis=0)
    return out.reshape(B, S, D_OUT), res


def kernel(x, weight, bias):
    out, _ = run(x, weight, bias)
    return out

